# revision 1
# baseline (speedup 1.0000x reference)
"""Trainium2 Bass kernel for nn_DeformKernelConv2d.

Math (per batch image; shapes below are per core after sharding):
  offsets:  off = conv3x3(x, offset_w) + offset_b          -> dy,dx per (k, pixel)
  coords:   yc_k = dy_k + by_k ; xc_k = dx_k + bx_k        (scope-kernel space)
  phi:      phi_y[k,i] = relu(1-|yc_k - i|), i=0..3        (likewise phi_x)
  Phi:      Phi_k[4*yi+xi] = phi_y[k,yi] * phi_x[k,xi]     (bilinear weights, 16 per k)
  samp:     samp_k[c] = sum_s wflat[c,s] * Phi_k[s]        (matmul over s=16)
  out:      out[c] = sum_k samp_k[c] * x_k[c]              (x_k = 3x3-shifted x)

Device mapping:
  - 8 cores: (batch b, H-half); each core does 28 rows with a 1-row halo.
  - The offset conv is fused with the affine (coord - grid line i) expansion
    into one 9-tap accumulated matmul producing T[72, pix] (rows (k, axis, i)),
    with per-row bias = offset_b + base - i folded into the Abs activation.
  - phi on ScalarE (Abs then Relu), phi->Y/X row replication by DMA,
    Phi products + final MAC on VectorE, samp matmuls row-packed 4x on PE.
"""

import numpy as np
import ml_dtypes

B, C, H, W = 4, 128, 56, 56
HC = H // 2            # 28 rows per core
NPIX = HC * W          # 1568
CH = 7                 # chunk height (rows)
NCH = HC // CH         # 4 chunks
CHN = CH * W           # 392 columns per chunk
RA, RB = 58, 60        # padded row lengths: xbfA data at col 1, xbfB at col 2

_BF16 = ml_dtypes.bfloat16
_cache = {}


def _build_program(repeat=1):
    import concourse.tile as tile
    import concourse.mybir as mybir
    from concourse import bacc

    fp32 = mybir.dt.float32
    bf16 = mybir.dt.bfloat16
    AF = mybir.ActivationFunctionType

    nc = bacc.Bacc("TRN2", target_bir_lowering=False, debug=False, num_devices=8)
    xs_d = nc.dram_tensor("xs", [C, HC + 2, W], fp32, kind="ExternalInput")
    lhsT_d = nc.dram_tensor("lhsT", [C, 9 * 72], bf16, kind="ExternalInput")
    w4T_d = nc.dram_tensor("w4T", [C, C], bf16, kind="ExternalInput")
    bias_d = nc.dram_tensor("bias72", [72, 1], fp32, kind="ExternalInput")
    out_d = nc.dram_tensor("out", [C, HC, W], fp32, kind="ExternalOutput")

    with tile.TileContext(nc) as tc:
        with (
            tc.tile_pool(name="const", bufs=1) as cp,
            tc.tile_pool(name="work", bufs=1) as wp,
            tc.tile_pool(name="tmp", bufs=3) as tp,
            tc.tile_pool(name="psT", bufs=2, space="PSUM") as ppT,
            tc.tile_pool(name="psS", bufs=5, space="PSUM") as ppS,
        ):
            lhsT = cp.tile([C, 9 * 72], bf16)
            nc.sync.dma_start(lhsT[:], lhsT_d[:])
            w4T = cp.tile([C, C], bf16)
            nc.sync.dma_start(w4T[:], w4T_d[:])
            bias = cp.tile([72, 1], fp32)
            nc.sync.dma_start(bias[:], bias_d[:])

            xbfA = cp.tile([C, HC + 2, RA], bf16)
            xbfB = cp.tile([C, HC + 2, RB], bf16)
            nc.vector.memset(xbfA[:, :, 0:1], 0)
            nc.vector.memset(xbfA[:, :, 57:58], 0)
            nc.vector.memset(xbfB[:, :, 0:2], 0)
            nc.vector.memset(xbfB[:, :, 58:60], 0)

            phi = wp.tile([72, NCH, CHN], bf16)
            Ya = wp.tile([C, NCH, CHN], bf16)
            Xa = wp.tile([C, NCH, CHN], bf16)
            Yb = wp.tile([C, NCH, CHN], bf16)
            Xb = wp.tile([C, NCH, CHN], bf16)
            Yc = wp.tile([16, NCH, CHN], bf16)
            Xc = wp.tile([16, NCH, CHN], bf16)
            PhA = wp.tile([C, NCH, CHN], bf16)
            PhB = wp.tile([C, NCH, CHN], bf16)
            PhC = wp.tile([16, NCH, CHN], bf16)
            samp = wp.tile([C, 9, NPIX], bf16)
            prod = wp.tile([C, 9, NPIX], bf16)
            t1 = wp.tile([C, 4, NPIX], bf16)
            t2 = wp.tile([C, 2, NPIX], bf16)
            t3 = wp.tile([C, NPIX], bf16)
            res = wp.tile([C, NPIX], fp32)

            # replication views (single strided partition dim per DMA side —
            # multi-dim partition APs mislower in the DMA path)
            phiV = phi[:].rearrange("(k a i) c n -> k a i c n", k=9, a=2, i=4)

            def rep_view(t):
                return t[:].rearrange("(g h y x) c n -> g h y x c n", h=2, y=4, x=4)

            YaV, XaV, YbV, XbV = rep_view(Ya), rep_view(Xa), rep_view(Yb), rep_view(Xb)
            YcV = Yc[:].rearrange("(y x) c n -> y x c n", x=4)
            XcV = Xc[:].rearrange("(y x) c n -> y x c n", x=4)

            for _rep in range(repeat):
                nc.gpsimd.dma_start(xbfA[:, :, 1:57], xs_d[:])  # casts fp32->bf16
                nc.gpsimd.dma_start(xbfB[:, :, 2:58], xs_d[:])

                # ---- T matmul (offset conv + affine expansion) + phi ----
                for ch in range(NCH):
                    psT = ppT.tile([72, CHN], fp32, tag="psT")
                    for tap in range(9):
                        di, dj = tap // 3, tap % 3
                        rhs = xbfA[:, ch * CH + di : ch * CH + di + CH, dj : dj + W]
                        nc.tensor.matmul(
                            psT[:],
                            lhsT[:, tap * 72 : (tap + 1) * 72],
                            rhs,
                            start=(tap == 0),
                            stop=(tap == 8),
                        )
                    u = tp.tile([72, CHN], fp32, tag="u")
                    nc.scalar.activation(u[:], psT[:], AF.Abs, bias=bias[:], scale=1.0)
                    nc.scalar.activation(
                        phi[:, ch, :], u[:], AF.Relu, bias=1.0, scale=-1.0
                    )

                # ---- replicate phi rows into Y/X s-patterns (72 DMAs) ----
                for g in range(4):
                    for xi in range(4):
                        nc.sync.dma_start(YaV[g, 0, :, xi], phiV[g, 0, :])
                        nc.sync.dma_start(YbV[g, 0, :, xi], phiV[4 + g, 0, :])
                    for yi in range(4):
                        nc.sync.dma_start(XaV[g, 0, yi, :], phiV[g, 1, :])
                        nc.sync.dma_start(XbV[g, 0, yi, :], phiV[4 + g, 1, :])
                for xi in range(4):
                    nc.sync.dma_start(YcV[:, xi], phiV[8, 0, :])
                for yi in range(4):
                    nc.sync.dma_start(XcV[yi, :], phiV[8, 1, :])

                # ---- Phi products, samp matmuls (row-packed), PSUM drain ----
                for ch in range(NCH):
                    nc.vector.tensor_mul(PhA[:, ch, :], Ya[:, ch, :], Xa[:, ch, :])
                    nc.vector.tensor_mul(PhB[:, ch, :], Yb[:, ch, :], Xb[:, ch, :])
                    nc.vector.tensor_mul(PhC[:, ch, :], Yc[:, ch, :], Xc[:, ch, :])
                    for k in range(9):
                        g = k % 4
                        src = (PhA, PhB, PhC)[k // 4]
                        base = 32 * g if k < 8 else 0
                        psS = ppS.tile([C, CHN], fp32, tag="psS")
                        nc.tensor.matmul(
                            psS[:],
                            w4T[base : base + 16, :],
                            src[base : base + 16, ch, :],
                            start=True,
                            stop=True,
                            tile_position=(base, 0),
                        )
                        dst = samp[:, k, ch * CHN : (ch + 1) * CHN]
                        if k < 6:
                            nc.scalar.copy(dst, psS[:])
                        else:
                            nc.vector.tensor_copy(out=dst, in_=psS[:])

                # ---- products with shifted x, tree sum, store ----
                for k in range(9):
                    di, dj = k // 3, k % 3
                    if dj == 1:
                        xsrc, coff = xbfB, dj + 1  # col offset 2: 4B aligned
                    else:
                        xsrc, coff = xbfA, dj  # col offsets 0, 2
                    xv = xsrc[:, di : di + HC, coff : coff + W]
                    nc.vector.tensor_mul(
                        prod[:, k, :].rearrange("p (h w) -> p h w", h=HC),
                        samp[:, k, :].rearrange("p (h w) -> p h w", h=HC),
                        xv,
                    )
                nc.vector.tensor_add(t1[:], prod[:, 0:4, :], prod[:, 4:8, :])
                nc.vector.tensor_add(t2[:], t1[:, 0:2, :], t1[:, 2:4, :])
                nc.vector.tensor_add(t3[:], t2[:, 0, :], t2[:, 1, :])
                nc.vector.tensor_add(res[:], t3[:], prod[:, 8, :])
                nc.sync.dma_start(
                    out_d[:], res[:].rearrange("p (h w) -> p h w", h=HC)
                )

    nc.finalize()
    return nc


def _prep_inputs(x, offset_w, offset_b, weight):
    """Host-side sharding + weight reshaping. Returns per-core input maps."""
    x = np.asarray(x, dtype=np.float32)
    offset_w = np.asarray(offset_w, dtype=np.float32)
    offset_b = np.asarray(offset_b, dtype=np.float32)
    weight = np.asarray(weight, dtype=np.float32)

    # lhsT[c, tap*72 + k*8 + axis*4 + i] = offset_w[2k+axis, c, tap//3, tap%3]
    ow = offset_w.reshape(9, 2, C, 3, 3)  # [k, axis, c, di, dj]
    lhsT = np.transpose(ow, (2, 3, 4, 0, 1))  # [c, di, dj, k, axis]
    lhsT = np.repeat(lhsT[..., None], 4, axis=-1)  # [c, di, dj, k, axis, i]
    lhsT = np.ascontiguousarray(lhsT.reshape(C, 648)).astype(_BF16)

    # w4T rows 32g+s = weight[:, s//4, s%4]
    w4T = np.zeros((C, C), dtype=_BF16)
    wT = weight.reshape(C, 16).T.astype(_BF16)  # [16, C]
    for g in range(4):
        w4T[32 * g : 32 * g + 16, :] = wT

    # bias72[k*8+axis*4+i] = offset_b[2k+axis] + base - i
    base = np.arange(3, dtype=np.float32) + 0.5
    bias = np.zeros((9, 2, 4), dtype=np.float32)
    for k in range(9):
        for axis in range(2):
            bv = base[k // 3] if axis == 0 else base[k % 3]
            bias[k, axis, :] = offset_b[2 * k + axis] + bv - np.arange(4)
    bias72 = bias.reshape(72, 1)

    in_maps = []
    for core in range(8):
        b, half = core // 2, core % 2
        h0 = half * HC
        xs = np.zeros((C, HC + 2, W), dtype=np.float32)
        lo, hi = h0 - 1, h0 + HC + 1
        slo, shi = max(lo, 0), min(hi, H)
        xs[:, slo - lo : slo - lo + (shi - slo), :] = x[b, :, slo:shi, :]
        in_maps.append({"xs": xs, "lhsT": lhsT, "w4T": w4T, "bias72": bias72})
    return in_maps


def kernel(x, offset_w, offset_b, weight):
    from concourse.bass_utils import run_bass_kernel_spmd

    if "nc" not in _cache:
        _cache["nc"] = _build_program()
    nc = _cache["nc"]

    in_maps = _prep_inputs(x, offset_w, offset_b, weight)
    res = run_bass_kernel_spmd(nc, in_maps, core_ids=list(range(8)))

    out = np.zeros((B, C, H, W), dtype=np.float32)
    for core in range(8):
        b, half = core // 2, core % 2
        out[b, :, half * HC : (half + 1) * HC, :] = res.results[core]["out"].reshape(
            C, HC, W
        )
    return out



# revision 5
# speedup vs baseline: 1.8157x; 1.8157x over previous
"""Trainium2 Bass kernel for nn_DeformKernelConv2d.

Math (per batch image; shapes below are per core after sharding):
  offsets:  off = conv3x3(x, offset_w) + offset_b          -> dy,dx per (k, pixel)
  coords:   yc_k = dy_k + by_k ; xc_k = dx_k + bx_k        (scope-kernel space)
  phi:      phi_y[k,i] = relu(1-|yc_k - i|), i=0..3        (likewise phi_x)
  Phi:      Phi_k[4*yi+xi] = phi_y[k,yi] * phi_x[k,xi]     (bilinear weights, 16 per k)
  samp:     samp_k[c] = sum_s wflat[c,s] * Phi_k[s]        (matmul over s=16)
  out:      out[c] = sum_k samp_k[c] * x_k[c]              (x_k = 3x3-shifted x)

Device mapping (v2 — DMA-light):
  - 8 cores: (batch b, H-half); each core does 28 rows with a 1-row halo.
  - Input arrives pre-padded bf16 [C, 30, 118]: copy1 at cols 1..56,
    copy2 at cols 59..114, so every 3x3-shifted view has a 4B-aligned copy.
  - Offset conv fused with affine expansion: one 9-tap accumulated matmul
    producing psT[72, pix] (rows (k, axis, i)); ACT Abs(+bias) then Relu
    gives phi[72] (the tent weights).
  - phi -> (g,h,s)-layout replication via 0/1 SELECTION MATMULS on the PE
    (replaces 72 SBUF-to-SBUF DMAs): psYA/psXA [128, pix], psYC/psXC [16].
  - Phi products on DVE, samp matmuls with zero-padded 32-row weight
    blocks (rhs/lhsT/tile_position all 32-aligned), per-k drain on ACT or
    fused PSUM multiply on DVE, tree-sum, bf16 store.
"""

import numpy as np
import ml_dtypes

B, C, H, W = 4, 128, 56, 56
HC = H // 2            # 28 rows per core
NPIX = HC * W          # 1568
CH = 7                 # chunk height (rows)
NCH = HC // CH         # 4 chunks
CHN = CH * W           # 392 columns per chunk
XW = 118               # padded input row: copy1 data @1..56, copy2 data @60..115

# blob column layout
O_LHST = 0             # [128, 648]  conv lhsT: tap*72 + (k*8 + a*4 + i)
O_SELYA = 648          # [72, 128]   selection (k,0,i) -> (g,h,s): i==s//4
O_SELXA = 776          # [72, 128]   selection (k,1,i) -> (g,h,s): i==s%4
O_SELYC = 904          # [72, 16]    k=8 y-rows
O_SELXC = 920          # [72, 16]    k=8 x-rows
O_WREP = 936           # [128, 256]  zero-padded 32-row weight blocks (h=0,1)
NBLOB = 1192

# which k's fuse the PSUM multiply on DVE (rest drain via ACT then 2x mul)
FUSED_K = (2, 5)

_BF16 = ml_dtypes.bfloat16
_cache = {}


def _build_program():
    import concourse.tile as tile
    import concourse.mybir as mybir
    from concourse import bacc

    fp32 = mybir.dt.float32
    bf16 = mybir.dt.bfloat16
    AF = mybir.ActivationFunctionType

    nc = bacc.Bacc("TRN2", target_bir_lowering=False, debug=False, num_devices=8)
    xs_d = nc.dram_tensor("xs", [C, HC + 2, XW], bf16, kind="ExternalInput")
    blob_d = nc.dram_tensor("blob", [C, NBLOB], bf16, kind="ExternalInput")
    bias_d = nc.dram_tensor("bias72", [72, 1], fp32, kind="ExternalInput")
    out_d = nc.dram_tensor("out", [C, HC, W], bf16, kind="ExternalOutput")

    with tile.TileContext(nc) as tc:
        with (
            tc.tile_pool(name="const", bufs=1) as cp,
            tc.tile_pool(name="work", bufs=1) as wp,
            tc.tile_pool(name="u", bufs=2) as up,
            tc.tile_pool(name="ph", bufs=2) as php,
            tc.tile_pool(name="sp", bufs=2) as sp,
            tc.tile_pool(name="psT", bufs=1, space="PSUM") as ppT,
            tc.tile_pool(name="psYA", bufs=1, space="PSUM") as ppYA,
            tc.tile_pool(name="psXA", bufs=1, space="PSUM") as ppXA,
            tc.tile_pool(name="psYC", bufs=1, space="PSUM") as ppYC,
            tc.tile_pool(name="psXC", bufs=1, space="PSUM") as ppXC,
            tc.tile_pool(name="psS", bufs=3, space="PSUM") as ppS,
        ):
            blob = cp.tile([C, NBLOB], bf16)
            bias = cp.tile([72, 1], fp32)
            xbig = cp.tile([C, HC + 2, XW], bf16)
            nc.scalar.dma_start(blob[:], blob_d[:])
            nc.scalar.dma_start(bias[:], bias_d[:])
            nc.sync.dma_start(xbig[:, 0:16, :], xs_d[:, 0:16, :])
            nc.sync.dma_start(xbig[:, 16:30, :], xs_d[:, 16:30, :])

            phi = wp.tile([72, NCH, CHN], bf16)

            for ch in range(NCH):
                # ---- offset conv + affine expansion -> phi[72] ----
                psT = ppT.tile([72, CHN], fp32, tag="psT")
                for tap in range(9):
                    di, dj = tap // 3, tap % 3
                    rhs = xbig[:, ch * CH + di : ch * CH + di + CH, dj : dj + W]
                    nc.tensor.matmul(
                        psT[:],
                        blob[:, O_LHST + tap * 72 : O_LHST + (tap + 1) * 72],
                        rhs,
                        start=(tap == 0),
                        stop=(tap == 8),
                    )
                u = up.tile([72, CHN], fp32, tag="u")
                nc.scalar.activation(u[:], psT[:], AF.Abs, bias=bias[:], scale=1.0)
                nc.scalar.activation(phi[:, ch, :], u[:], AF.Relu, bias=1.0, scale=-1.0)
                rphi = phi[:, ch, :]

                # ---- replicate phi into (g,h,s) layout via selection matmuls ----
                psYA = ppYA.tile([C, CHN], fp32, tag="psYA")
                nc.tensor.matmul(psYA[:], blob[0:72, O_SELYA : O_SELYA + 128], rphi)
                psXA = ppXA.tile([C, CHN], fp32, tag="psXA")
                nc.tensor.matmul(psXA[:], blob[0:72, O_SELXA : O_SELXA + 128], rphi)
                psYC = ppYC.tile([16, CHN], fp32, tag="psYC")
                nc.tensor.matmul(psYC[:], blob[0:72, O_SELYC : O_SELYC + 16], rphi)
                psXC = ppXC.tile([16, CHN], fp32, tag="psXC")
                nc.tensor.matmul(psXC[:], blob[0:72, O_SELXC : O_SELXC + 16], rphi)

                YA = php.tile([C, CHN], bf16, tag="YA")
                XA = php.tile([C, CHN], bf16, tag="XA")
                YC = php.tile([16, CHN], bf16, tag="YC")
                XC = php.tile([16, CHN], bf16, tag="XC")
                nc.scalar.copy(YA[:], psYA[:])
                nc.scalar.copy(XA[:], psXA[:])
                nc.scalar.copy(YC[:], psYC[:])
                nc.scalar.copy(XC[:], psXC[:])

                PhAB = php.tile([C, CHN], bf16, tag="PhAB")
                PhC = php.tile([16, CHN], bf16, tag="PhC")
                nc.vector.tensor_mul(PhAB[:], YA[:], XA[:])
                nc.vector.tensor_mul(PhC[:], YC[:], XC[:])

                # ---- samp matmuls + per-k drain/multiply ----
                samp = sp.tile([C, 9, CHN], bf16, tag="samp")
                prod = sp.tile([C, 9, CHN], bf16, tag="prod")
                for k in range(9):
                    psS = ppS.tile([C, CHN], fp32, tag="psS")
                    if k < 8:
                        g, h = k % 4, k // 4
                        nc.tensor.matmul(
                            psS[:],
                            blob[32 * g : 32 * g + 32, O_WREP + 128 * h : O_WREP + 128 * h + 128],
                            PhAB[32 * g : 32 * g + 32, :],
                            tile_position=(32 * g, 0),
                        )
                    else:
                        nc.tensor.matmul(
                            psS[:],
                            blob[0:16, O_WREP : O_WREP + 128],
                            PhC[:],
                            tile_position=(0, 0),
                        )
                    di, dj = k // 3, k % 3
                    # window(dj) reads x[w+dj-1]: copy1 (data@1) starts at
                    # col dj (even for dj=0,2); copy2 (data@60) starts at 60
                    # for dj=1. All drained-k views are 4B-aligned.
                    c0 = dj if dj != 1 else 60
                    xv = xbig[:, ch * CH + di : ch * CH + di + CH, c0 : c0 + W]
                    pv = prod[:, k, :].rearrange("p (h w) -> p h w", h=CH)
                    if k in FUSED_K:
                        nc.vector.tensor_mul(
                            pv, psS[:].rearrange("p (h w) -> p h w", h=CH), xv
                        )
                    else:
                        nc.scalar.copy(samp[:, k, :], psS[:])
                        nc.vector.tensor_mul(
                            pv,
                            samp[:, k, :].rearrange("p (h w) -> p h w", h=CH),
                            xv,
                        )

                # ---- tree sum + store ----
                t1 = sp.tile([C, 4, CHN], bf16, tag="t1")
                t2 = sp.tile([C, 2, CHN], bf16, tag="t2")
                t3 = sp.tile([C, CHN], bf16, tag="t3")
                res = sp.tile([C, CHN], bf16, tag="res")
                nc.vector.tensor_add(t1[:], prod[:, 0:4, :], prod[:, 4:8, :])
                nc.vector.tensor_add(t2[:], t1[:, 0:2, :], t1[:, 2:4, :])
                nc.vector.tensor_add(t3[:], t2[:, 0, :], t2[:, 1, :])
                nc.vector.tensor_add(res[:], t3[:], prod[:, 8, :])
                nc.sync.dma_start(
                    out_d[:, ch * CH : (ch + 1) * CH, :],
                    res[:].rearrange("p (h w) -> p h w", h=CH),
                )

    nc.finalize()
    return nc


def _prep_inputs(x, offset_w, offset_b, weight):
    """Host-side sharding + weight reshaping. Returns per-core input maps."""
    x = np.asarray(x, dtype=np.float32)
    offset_w = np.asarray(offset_w, dtype=np.float32)
    offset_b = np.asarray(offset_b, dtype=np.float32)
    weight = np.asarray(weight, dtype=np.float32)

    blob = np.zeros((C, NBLOB), dtype=np.float32)

    # conv lhsT[c, tap*72 + k*8 + axis*4 + i] = offset_w[2k+axis, c, di, dj]
    ow = offset_w.reshape(9, 2, C, 3, 3)  # [k, axis, c, di, dj]
    lhsT = np.transpose(ow, (2, 3, 4, 0, 1))  # [c, di, dj, k, axis]
    lhsT = np.repeat(lhsT[..., None], 4, axis=-1)  # [c, di, dj, k, axis, i]
    blob[:, O_LHST : O_LHST + 648] = lhsT.reshape(C, 648)

    # selection matrices: row r=(k,a,i) -> col p=(g,h,s), s=(yi,xi)
    selYA = np.zeros((72, 128), dtype=np.float32)
    selXA = np.zeros((72, 128), dtype=np.float32)
    selYC = np.zeros((72, 16), dtype=np.float32)
    selXC = np.zeros((72, 16), dtype=np.float32)
    for g in range(4):
        for h in range(2):
            k = g + 4 * h
            for s in range(16):
                p = 32 * g + 16 * h + s
                selYA[k * 8 + 0 * 4 + s // 4, p] = 1.0
                selXA[k * 8 + 1 * 4 + s % 4, p] = 1.0
    for s in range(16):
        selYC[8 * 8 + 0 * 4 + s // 4, s] = 1.0
        selXC[8 * 8 + 1 * 4 + s % 4, s] = 1.0
    blob[0:72, O_SELYA : O_SELYA + 128] = selYA
    blob[0:72, O_SELXA : O_SELXA + 128] = selXA
    blob[0:72, O_SELYC : O_SELYC + 16] = selYC
    blob[0:72, O_SELXC : O_SELXC + 16] = selXC

    # zero-padded 32-row weight blocks: partition 32g+s2, col h*128+c
    wT = weight.reshape(C, 16).T  # [16, C]
    wrep = np.zeros((C, 256), dtype=np.float32)
    for g in range(4):
        wrep[32 * g : 32 * g + 16, 0:128] = wT
        wrep[32 * g + 16 : 32 * g + 32, 128:256] = wT
    blob[:, O_WREP : O_WREP + 256] = wrep
    blob = blob.astype(_BF16)

    # bias72[k*8+axis*4+i] = offset_b[2k+axis] + base - i
    base = np.arange(3, dtype=np.float32) + 0.5
    bias = np.zeros((9, 2, 4), dtype=np.float32)
    for k in range(9):
        for axis in range(2):
            bv = base[k // 3] if axis == 0 else base[k % 3]
            bias[k, axis, :] = offset_b[2 * k + axis] + bv - np.arange(4)
    bias72 = bias.reshape(72, 1)

    xbf = x.astype(_BF16)
    in_maps = []
    for core in range(8):
        b, half = core // 2, core % 2
        h0 = half * HC
        xs = np.zeros((C, HC + 2, XW), dtype=_BF16)
        lo, hi = h0 - 1, h0 + HC + 1
        slo, shi = max(lo, 0), min(hi, H)
        rows = xbf[b, :, slo:shi, :]
        xs[:, slo - lo : slo - lo + (shi - slo), 1:57] = rows
        xs[:, slo - lo : slo - lo + (shi - slo), 60:116] = rows
        in_maps.append({"xs": xs, "blob": blob, "bias72": bias72})
    return in_maps


def kernel(x, offset_w, offset_b, weight):
    from concourse.bass_utils import run_bass_kernel_spmd

    if "nc" not in _cache:
        _cache["nc"] = _build_program()
    nc = _cache["nc"]

    in_maps = _prep_inputs(x, offset_w, offset_b, weight)
    res = run_bass_kernel_spmd(nc, in_maps, core_ids=list(range(8)))

    out = np.zeros((B, C, H, W), dtype=np.float32)
    for core in range(8):
        b, half = core // 2, core % 2
        out[b, :, half * HC : (half + 1) * HC, :] = np.asarray(
            res.results[core]["out"], dtype=np.float32
        ).reshape(C, HC, W)
    return out
